# revision 32
# baseline (speedup 1.0000x reference)
"""AttentionBlock (GroupNorm -> qkv conv1x1 -> 4-head attention -> proj + residual)
on 8 Trainium2 NeuronCores.

Sharding: B*NH = 2*4 = 8 (batch, head) pairs -> one per core.
Each core:
  - GroupNorm(32, 512) over its batch's x (recomputed per core)
  - qkv for its head:  q,k,v = W'[3*128, 512] @ xn   (norm affine + qk scale
    folded into W'/bias on host)
  - scoresT[s,t] = sum_c k[c,s] q[c,t]  (s on partitions -> exp output needs
    no transposes).  No max-subtraction: scores are O(1) for this problem.
  - eT = exp(scoresT) (bf16);  Z[t] via fp16 pairwise add-tree + ones-matmul
  - h_unnorm[c,t] = sum_s v[c,s] eT[s,t]
  - partial[o,t] = w_proj[o, head_slice] @ h_unnorm ; Z shipped to host
Host: out[b] = sum_heads partial/Z + b_proj + x  (gather/unshard).

Pipeline: rounds r=0..4; round r interleaves scores+exp of chunk r with the
attn@v accumulation of chunk r-1 at s-tile granularity so the scalar engine
(exp) never starves while the PE does attn@v / proj.
"""

import math
from contextlib import ExitStack

import ml_dtypes
import numpy as np

import concourse.bacc as bacc
import concourse.bass as bass
import concourse.mybir as mybir
import concourse.tile as tile
from concourse.bass_utils import run_bass_kernel_spmd

C = 512
NH = 4
G = 32
EPS = 1e-5
N = 4096          # H*W
CH = 128          # channels per head
B = 2
NCORES = 8
TCHUNK = 1024     # t-columns processed per chunk
NCHUNK = N // TCHUNK
NST = N // 128    # number of 128-wide s tiles

F16 = mybir.dt.float16
BF16 = mybir.dt.bfloat16
F32 = mybir.dt.float32

TRACE = False
TRACE_CORES = [0]
LAST_RESULT = None


def build_program():
    nc = bacc.Bacc()

    x16 = nc.declare_dram_parameter("x16", [C, N], BF16, isOutput=False)
    wqkvT = nc.declare_dram_parameter("wqkvT", [4, 128, 3 * CH], BF16, isOutput=False)
    bqkv = nc.declare_dram_parameter("bqkv", [128, 3], F32, isOutput=False)
    wprojT = nc.declare_dram_parameter("wprojT", [CH, C], BF16, isOutput=False)
    # group membership matrices: mgrp[p, g] = (p // 16 == g)
    mgrp = nc.declare_dram_parameter("mgrp", [128, 8], BF16, isOutput=False)
    mgrpT = nc.declare_dram_parameter("mgrpT", [8, 128], BF16, isOutput=False)
    partial = nc.declare_dram_parameter("partial", [C, N], F32, isOutput=True)
    zout = nc.declare_dram_parameter("zout", [1, N], F32, isOutput=True)

    with tile.TileContext(nc) as tc, ExitStack() as ctx:
        consts = ctx.enter_context(tc.tile_pool(name="consts", bufs=1))
        gn = ctx.enter_context(tc.tile_pool(name="gn", bufs=1))
        xpool = ctx.enter_context(tc.tile_pool(name="xpool", bufs=4))
        spool = ctx.enter_context(tc.tile_pool(name="spool", bufs=2))
        qkvp = ctx.enter_context(tc.tile_pool(name="qkvp", bufs=1))
        epool = ctx.enter_context(tc.tile_pool(name="epool", bufs=17))
        trpool = ctx.enter_context(tc.tile_pool(name="trpool", bufs=8))
        espool = ctx.enter_context(tc.tile_pool(name="espool", bufs=2))
        zpool = ctx.enter_context(tc.tile_pool(name="zpool", bufs=1))
        hpool = ctx.enter_context(tc.tile_pool(name="hpool", bufs=3))
        opool = ctx.enter_context(tc.tile_pool(name="opool", bufs=3))
        ps_sc = ctx.enter_context(tc.tile_pool(name="ps_sc", bufs=2, space="PSUM"))
        ps_acc = ctx.enter_context(tc.tile_pool(name="ps_acc", bufs=2, space="PSUM"))
        ps_mm2 = ctx.enter_context(tc.tile_pool(name="ps_mm2", bufs=2, space="PSUM"))

        # ---- constants ----
        mgrp_sb = consts.tile([128, 8], BF16, tag="mgrp")
        nc.sync.dma_start(out=mgrp_sb, in_=mgrp[:, :])
        mgrpT_sb = consts.tile([8, 128], BF16, tag="mgrpT")
        nc.sync.dma_start(out=mgrpT_sb, in_=mgrpT[:, :])
        ones_col = consts.tile([128, 1], F16, tag="ones")
        nc.vector.memset(ones_col, 1.0)
        eps_sb = consts.tile([128, 1], F32, tag="eps")
        nc.vector.memset(eps_sb, EPS)

        w_tiles = []
        for kt in range(4):
            wt = consts.tile([128, 3 * CH], BF16, tag=f"wq{kt}")
            nc.sync.dma_start(out=wt, in_=wqkvT[kt])
            w_tiles.append(wt)
        bq_sb = consts.tile([128, 3], F32, tag="bq")
        nc.sync.dma_start(out=bq_sb, in_=bqkv[:, :])
        wp_sb = consts.tile([CH, C], BF16, tag="wp")
        nc.sync.dma_start(out=wp_sb, in_=wprojT[:, :])

        # ---- load x tiles + per-channel stats ----
        # tiles 0,1: vector bn_stats; tiles 2,3: scalar Square/Identity accum
        stats_all = gn.tile([128, 8], F32, tag="stats_all")
        xt = []
        for i in range(4):
            xti = xpool.tile([128, N], BF16, tag="xt")
            dma_eng = nc.sync if i % 2 == 0 else nc.scalar
            dma_eng.dma_start(out=xti, in_=x16[128 * i : 128 * (i + 1), :])
            xt.append(xti)
            if i < 2:
                st = spool.tile([128, 8, 6], F32, tag="bst")
                xv = xti.rearrange("p (s f) -> p s f", f=512)
                for s in range(8):
                    nc.vector.bn_stats(out=st[:, s, :], in_=xv[:, s, :])
                mv = spool.tile([128, 2], F32, tag="mv")
                nc.vector.bn_aggr(out=mv, in_=st)
                # stats_all[:, i] = channel mean;  stats_all[:, 4+i] = E[x^2]
                nc.vector.tensor_copy(out=stats_all[:, i : i + 1], in_=mv[:, 0:1])
                nc.vector.tensor_mul(
                    out=stats_all[:, 4 + i : 5 + i], in0=mv[:, 0:1], in1=mv[:, 0:1]
                )
                nc.vector.tensor_add(
                    out=stats_all[:, 4 + i : 5 + i],
                    in0=stats_all[:, 4 + i : 5 + i],
                    in1=mv[:, 1:2],
                )
            else:
                sq_scr = qkvp.tile([128, N], BF16, tag="qkv0", name=f"sq_scr{i}")
                sx2 = spool.tile([128, 1], F32, tag="sx2", name=f"sx2_{i}")
                nc.scalar.activation(
                    out=sq_scr,
                    in_=xti,
                    func=mybir.ActivationFunctionType.Square,
                    accum_out=sx2,
                )
                sx1 = spool.tile([128, 1], F32, tag="sx1", name=f"sx1_{i}")
                nc.scalar.activation(
                    out=xti,
                    in_=xti,
                    func=mybir.ActivationFunctionType.Identity,
                    accum_out=sx1,
                )
                nc.vector.tensor_scalar_mul(
                    out=stats_all[:, i : i + 1], in0=sx1, scalar1=1.0 / N
                )
                nc.vector.tensor_scalar_mul(
                    out=stats_all[:, 4 + i : 5 + i], in0=sx2, scalar1=1.0 / N
                )

        # ---- cross-partition group aggregation via PE ----
        stats16 = gn.tile([128, 8], BF16, tag="stats16")
        nc.vector.tensor_copy(out=stats16, in_=stats_all)
        ps_t = ps_mm2.tile([8, 8], F32, tag="mm2")
        nc.tensor.matmul(ps_t, lhsT=mgrp_sb, rhs=stats16, start=True, stop=True)
        gs = gn.tile([8, 8], F32, tag="gs8")
        nc.scalar.mul(out=gs, in_=ps_t, mul=1.0 / 16.0)
        # gvals cols 0..3 = group mean per x-tile, cols 4..7 = group rstd
        gvals = gn.tile([8, 8], F32, tag="gvals")
        nc.vector.tensor_copy(out=gvals[:, 0:4], in_=gs[:, 0:4])
        varg = gn.tile([8, 4], F32, tag="varg")
        nc.vector.tensor_mul(out=varg, in0=gs[:, 0:4], in1=gs[:, 0:4])  # mu^2
        nc.vector.tensor_sub(out=varg, in0=gs[:, 4:8], in1=varg)  # var
        nc.scalar.activation(
            out=varg,
            in_=varg,
            func=mybir.ActivationFunctionType.Sqrt,
            bias=eps_sb[0:8, :],
        )
        nc.vector.reciprocal(out=gvals[:, 4:8], in_=varg)  # rstd
        gvals16 = gn.tile([8, 8], BF16, tag="gvals16")
        nc.vector.tensor_copy(out=gvals16, in_=gvals)
        ps_t2 = ps_mm2.tile([128, 8], F32, tag="mm2")
        nc.tensor.matmul(ps_t2, lhsT=mgrpT_sb, rhs=gvals16, start=True, stop=True)
        sc_all = gn.tile([128, 8], F32, tag="scall")
        nc.vector.tensor_copy(out=sc_all, in_=ps_t2)

        # ---- fold normalization into the weights instead of applying to x:
        # qkv = (W' * rstd[cin]) @ x_raw + (b' - (W'*rstd) @ mu) ----
        wts = []
        nmu = gn.tile([128, 4], BF16, tag="nmu")
        for kt in range(4):
            wt_s = qkvp.tile([128, 3 * CH], BF16, tag=f"wts{kt}", name=f"wts{kt}")
            nc.vector.tensor_scalar_mul(
                out=wt_s, in0=w_tiles[kt], scalar1=sc_all[:, 4 + kt : 5 + kt]
            )
            wts.append(wt_s)
            nc.vector.tensor_scalar_mul(
                out=nmu[:, kt : kt + 1],
                in0=sc_all[:, kt : kt + 1],
                scalar1=-1.0,
            )
        beff = gn.tile([128, 3], F32, tag="beff")
        for j in range(3):
            ps_b = ps_mm2.tile([128, 1], F32, tag="mm2", name=f"ps_b{j}")
            for kt in range(4):
                nc.tensor.matmul(
                    ps_b,
                    lhsT=wts[kt][:, j * 128 : (j + 1) * 128],
                    rhs=nmu[:, kt : kt + 1],
                    start=(kt == 0),
                    stop=(kt == 3),
                )
            nc.vector.tensor_add(
                out=beff[:, j : j + 1], in0=bq_sb[:, j : j + 1], in1=ps_b
            )

        # ---- qkv = W' @ xn + b', chunk-major, v first so vT transposes
        # (serial 1.3us DMA-xbar ops) start as early as possible ----
        qkv_sb = [None, None, None]
        for j in range(3):
            qkv_sb[j] = qkvp.tile([128, N], BF16, tag=f"qkv{j}", name=f"qkv{j}")
        q_sb, k_sb, v_sb = qkv_sb
        vT = qkvp.tile([128, NST, 128], BF16, tag="vT")

        def qkv_pair(j, c2):
            # 1024 output cols (two 512 psum banks), stationary shared
            # between the two halves per kt to amortize LDWEIGHTS
            ps = ps_sc.tile(
                [128, 1024], F32, tag="sc", name=f"qps{j}_{c2}"
            )
            for kt in range(4):
                for half in range(2):
                    nc.tensor.matmul(
                        ps[:, 512 * half : 512 * (half + 1)],
                        lhsT=wts[kt][:, j * 128 : (j + 1) * 128],
                        rhs=xt[kt][
                            :, 1024 * c2 + 512 * half : 1024 * c2 + 512 * (half + 1)
                        ],
                        start=(kt == 0),
                        stop=(kt == 3),
                    )
            nc.scalar.activation(
                out=qkv_sb[j][:, 1024 * c2 : 1024 * (c2 + 1)],
                in_=ps,
                func=mybir.ActivationFunctionType.Identity,
                bias=beff[:, j : j + 1],
            )

        # ---- pipelined rounds: scores+exp(r) interleaved with attn@v(r-1).
        # Round 0 also interleaves the qkv GEMMs and vT transposes so exp
        # starts as soon as the first k columns exist. ----
        qkv_pair(0, 0)  # q cols 0..1023 (all that chunk-0 scores need)
        ets_prev = None
        for r in range(NCHUNK + 1):
            t0 = r * TCHUNK
            tp = (r - 1) * TCHUNK

            if r >= 1:
                # Z add-tree for chunk r-1 over the 16 pair tiles, emitted up
                # front (vector runs it while PE+ACT stream the st loop);
                # FD=2048 ops, in-place reduction on 8 temps
                tt = []
                for j in range(8):
                    t_ = trpool.tile([128, 2, TCHUNK], F16, tag="trv", name=f"t{j}")
                    nc.vector.tensor_add(
                        out=t_, in0=ets_prev[2 * j], in1=ets_prev[2 * j + 1]
                    )
                    tt.append(t_)
                for span in (2, 4, 8):
                    for j in range(0, 8, span):
                        nc.vector.tensor_add(
                            out=tt[j], in0=tt[j], in1=tt[j + span // 2]
                        )
                ps_h = [
                    ps_acc.tile([128, 512], F32, tag="acc", name=f"ps_h{i}")
                    for i in range(2)
                ]

            ets = []
            for stt in range(NST):
                if r == 0 and stt % 8 == 0:
                    c2 = stt // 8
                    qkv_pair(2, c2)  # v columns for these s tiles
                    qkv_pair(1, c2)  # k columns for these s tiles
                    for st2 in range(8 * c2, 8 * c2 + 8):
                        eng = nc.sync if st2 % 2 == 0 else nc.scalar
                        eng.dma_start_transpose(
                            vT[:, st2, :], v_sb[:, 128 * st2 : 128 * (st2 + 1)]
                        )
                    if c2 < 3:
                        qkv_pair(0, c2 + 1)  # next q pair
                if r < NCHUNK:
                    ps = ps_sc.tile([128, TCHUNK], F32, tag="sc")
                    kslice = k_sb[:, 128 * stt : 128 * (stt + 1)]
                    for hh in range(2):
                        nc.tensor.matmul(
                            ps[:, 512 * hh : 512 * (hh + 1)],
                            lhsT=kslice,
                            rhs=q_sb[:, t0 + 512 * hh : t0 + 512 * (hh + 1)],
                            start=True,
                            stop=True,
                        )
                    if stt % 2 == 0:
                        et = epool.tile([128, 2, TCHUNK], BF16, tag="et")
                        ets.append(et)
                    nc.scalar.activation(
                        out=ets[stt // 2][:, stt % 2, :],
                        in_=ps,
                        func=mybir.ActivationFunctionType.Exp,
                    )
                if r >= 1:
                    ep = ets_prev[stt // 2]
                    for hh in range(2):
                        nc.tensor.matmul(
                            ps_h[hh],
                            lhsT=vT[:, stt, :],
                            rhs=ep[:, stt % 2, 512 * hh : 512 * (hh + 1)],
                            start=(stt == 0),
                            stop=(stt == NST - 1),
                        )

            if r >= 1:
                # finish Z tree, Z matmul, ship Z
                esum = espool.tile([128, TCHUNK], F16, tag="esum")
                nc.vector.tensor_add(out=esum, in0=tt[0][:, 0, :], in1=tt[0][:, 1, :])
                zrow = zpool.tile([1, TCHUNK], F32, tag="zrow")
                for hh in range(2):
                    ps_z = ps_mm2.tile([1, 512], F32, tag="mm2")
                    nc.tensor.matmul(
                        ps_z,
                        lhsT=ones_col,
                        rhs=esum[:, 512 * hh : 512 * (hh + 1)],
                        start=True,
                        stop=True,
                    )
                    nc.vector.tensor_copy(
                        out=zrow[:, 512 * hh : 512 * (hh + 1)], in_=ps_z
                    )
                nc.sync.dma_start(out=zout[:, tp : tp + TCHUNK], in_=zrow)

                # h_unnorm, proj, store
                for hh in range(2):
                    h_sb = hpool.tile([128, 512], BF16, tag="h")
                    nc.vector.tensor_copy(out=h_sb, in_=ps_h[hh])
                    for ot in range(4):
                        ps_p = ps_mm2.tile([128, 512], F32, tag="mm2")
                        nc.tensor.matmul(
                            ps_p,
                            lhsT=wp_sb[:, 128 * ot : 128 * (ot + 1)],
                            rhs=h_sb,
                            start=True,
                            stop=True,
                        )
                        ob = opool.tile([128, 512], F32, tag="osb")
                        nc.vector.tensor_copy(out=ob, in_=ps_p)
                        nc.sync.dma_start(
                            out=partial[
                                128 * ot : 128 * (ot + 1),
                                tp + 512 * hh : tp + 512 * (hh + 1),
                            ],
                            in_=ob,
                        )
            ets_prev = ets if r < NCHUNK else None

    if not nc.is_finalized():
        nc.finalize()
    return nc


_NC_CACHE = None


def _get_nc():
    global _NC_CACHE
    if _NC_CACHE is None:
        _NC_CACHE = build_program()
    return _NC_CACHE


def kernel(x, norm_w, norm_b, w_qkv, w_proj, b_proj):
    global LAST_RESULT
    x = np.asarray(x, dtype=np.float32)
    norm_w = np.asarray(norm_w, dtype=np.float32)
    norm_b = np.asarray(norm_b, dtype=np.float32)
    w_qkv = np.asarray(w_qkv, dtype=np.float32)
    w_proj = np.asarray(w_proj, dtype=np.float32)
    b_proj = np.asarray(b_proj, dtype=np.float32)

    s1 = 1.0 / math.sqrt(math.sqrt(CH))
    bf16 = ml_dtypes.bfloat16
    mgrp = (np.arange(128)[:, None] // 16 == np.arange(8)[None, :]).astype(bf16)
    in_maps = []
    for core in range(NCORES):
        b, h = divmod(core, NH)
        # reference layout: head h of batch b uses w_qkv rows
        # [384h:384h+128] (q), [384h+128:384h+256] (k), [384h+256:384h+384] (v)
        rows = w_qkv[384 * h : 384 * (h + 1)]  # (384, 512)
        wfold = rows * norm_w[None, :]  # fold GroupNorm gamma
        bias = rows @ norm_b  # fold GroupNorm beta
        scale_vec = np.concatenate(
            [np.full(128, s1), np.full(128, s1), np.ones(128)]
        ).astype(np.float32)
        wfold = wfold * scale_vec[:, None]
        bias = bias * scale_vec
        wqkvT = np.ascontiguousarray(wfold.T.reshape(4, 128, 384).astype(bf16))
        bqkv = np.ascontiguousarray(bias.reshape(3, 128).T.astype(np.float32))
        wprojT = np.ascontiguousarray(
            w_proj[:, 128 * h : 128 * (h + 1)].T.astype(bf16)
        )
        x16 = np.ascontiguousarray(x[b].reshape(C, N).astype(bf16))
        in_maps.append(
            {
                "x16": x16,
                "wqkvT": wqkvT,
                "bqkv": bqkv,
                "wprojT": wprojT,
                "mgrp": mgrp,
                "mgrpT": np.ascontiguousarray(mgrp.T),
            }
        )

    nc = _get_nc()
    res = run_bass_kernel_spmd(
        nc,
        in_maps,
        list(range(NCORES)),
        trace=TRACE,
        trace_cores=TRACE_CORES if TRACE else None,
    )
    LAST_RESULT = res

    out = np.empty((B, C, N), dtype=np.float32)
    for b in range(B):
        acc = x[b].reshape(C, N) + b_proj[:, None]
        for h in range(NH):
            r = res.results[4 * b + h]
            acc = acc + r["partial"] / r["zout"]
        out[b] = acc
    return out.reshape(B, C, 64, 64)


# revision 33
# speedup vs baseline: 1.0376x; 1.0376x over previous
"""AttentionBlock (GroupNorm -> qkv conv1x1 -> 4-head attention -> proj + residual)
on 8 Trainium2 NeuronCores.

Sharding: B*NH = 2*4 = 8 (batch, head) pairs -> one per core.
Each core:
  - GroupNorm(32, 512) over its batch's x (recomputed per core)
  - qkv for its head:  q,k,v = W'[3*128, 512] @ xn   (norm affine + qk scale
    folded into W'/bias on host)
  - scoresT[s,t] = sum_c k[c,s] q[c,t]  (s on partitions -> exp output needs
    no transposes).  No max-subtraction: scores are O(1) for this problem.
  - eT = exp(scoresT) (bf16);  Z[t] via fp16 pairwise add-tree + ones-matmul
  - h_unnorm[c,t] = sum_s v[c,s] eT[s,t]
  - partial[o,t] = w_proj[o, head_slice] @ h_unnorm ; Z shipped to host
Host: out[b] = sum_heads partial/Z + b_proj + x  (gather/unshard).

Pipeline: rounds r=0..4; round r interleaves scores+exp of chunk r with the
attn@v accumulation of chunk r-1 at s-tile granularity so the scalar engine
(exp) never starves while the PE does attn@v / proj.
"""

import math
from contextlib import ExitStack

import ml_dtypes
import numpy as np

import concourse.bacc as bacc
import concourse.bass as bass
import concourse.mybir as mybir
import concourse.tile as tile
from concourse.bass_utils import run_bass_kernel_spmd

C = 512
NH = 4
G = 32
EPS = 1e-5
N = 4096          # H*W
CH = 128          # channels per head
B = 2
NCORES = 8
TCHUNK = 1024     # t-columns processed per chunk
NCHUNK = N // TCHUNK
NST = N // 128    # number of 128-wide s tiles

F16 = mybir.dt.float16
BF16 = mybir.dt.bfloat16
F32 = mybir.dt.float32

TRACE = False
TRACE_CORES = [0]
LAST_RESULT = None


def build_program():
    nc = bacc.Bacc()

    x16 = nc.declare_dram_parameter("x16", [C, N], BF16, isOutput=False)
    wqkvT = nc.declare_dram_parameter("wqkvT", [4, 128, 3 * CH], BF16, isOutput=False)
    bqkv = nc.declare_dram_parameter("bqkv", [128, 3], F32, isOutput=False)
    wprojT = nc.declare_dram_parameter("wprojT", [CH, C], BF16, isOutput=False)
    # group membership matrices: mgrp[p, g] = (p // 16 == g)
    mgrp = nc.declare_dram_parameter("mgrp", [128, 8], BF16, isOutput=False)
    mgrpT = nc.declare_dram_parameter("mgrpT", [8, 128], BF16, isOutput=False)
    partial = nc.declare_dram_parameter("partial", [C, N], F32, isOutput=True)
    zout = nc.declare_dram_parameter("zout", [1, N], F32, isOutput=True)

    with tile.TileContext(nc) as tc, ExitStack() as ctx:
        consts = ctx.enter_context(tc.tile_pool(name="consts", bufs=1))
        gn = ctx.enter_context(tc.tile_pool(name="gn", bufs=1))
        xpool = ctx.enter_context(tc.tile_pool(name="xpool", bufs=4))
        spool = ctx.enter_context(tc.tile_pool(name="spool", bufs=2))
        qkvp = ctx.enter_context(tc.tile_pool(name="qkvp", bufs=1))
        epool = ctx.enter_context(tc.tile_pool(name="epool", bufs=17))
        trpool = ctx.enter_context(tc.tile_pool(name="trpool", bufs=8))
        espool = ctx.enter_context(tc.tile_pool(name="espool", bufs=2))
        zpool = ctx.enter_context(tc.tile_pool(name="zpool", bufs=1))
        hpool = ctx.enter_context(tc.tile_pool(name="hpool", bufs=3))
        opool = ctx.enter_context(tc.tile_pool(name="opool", bufs=3))
        ps_sc = ctx.enter_context(tc.tile_pool(name="ps_sc", bufs=2, space="PSUM"))
        ps_acc = ctx.enter_context(tc.tile_pool(name="ps_acc", bufs=2, space="PSUM"))
        ps_mm2 = ctx.enter_context(tc.tile_pool(name="ps_mm2", bufs=2, space="PSUM"))

        # ---- constants ----
        mgrp_sb = consts.tile([128, 8], BF16, tag="mgrp")
        nc.sync.dma_start(out=mgrp_sb, in_=mgrp[:, :])
        mgrpT_sb = consts.tile([8, 128], BF16, tag="mgrpT")
        nc.sync.dma_start(out=mgrpT_sb, in_=mgrpT[:, :])
        ones_col = consts.tile([128, 1], F16, tag="ones")
        nc.vector.memset(ones_col, 1.0)
        eps_sb = consts.tile([128, 1], F32, tag="eps")
        nc.vector.memset(eps_sb, EPS)

        w_tiles = []
        for kt in range(4):
            wt = consts.tile([128, 3 * CH], BF16, tag=f"wq{kt}")
            nc.sync.dma_start(out=wt, in_=wqkvT[kt])
            w_tiles.append(wt)
        bq_sb = consts.tile([128, 3], F32, tag="bq")
        nc.sync.dma_start(out=bq_sb, in_=bqkv[:, :])
        wp_sb = consts.tile([CH, C], BF16, tag="wp")
        nc.sync.dma_start(out=wp_sb, in_=wprojT[:, :])

        # ---- load x tiles + per-channel stats ----
        # tiles 0-2: vector bn_stats; tile 3: scalar Square/Identity accum
        stats_all = gn.tile([128, 8], F32, tag="stats_all")
        xt = []
        for i in range(4):
            xti = xpool.tile([128, N], BF16, tag="xt")
            dma_eng = nc.sync if i % 2 == 0 else nc.scalar
            dma_eng.dma_start(out=xti, in_=x16[128 * i : 128 * (i + 1), :])
            xt.append(xti)
            if i < 3:
                st = spool.tile([128, 8, 6], F32, tag="bst")
                xv = xti.rearrange("p (s f) -> p s f", f=512)
                for s in range(8):
                    nc.vector.bn_stats(out=st[:, s, :], in_=xv[:, s, :])
                mv = spool.tile([128, 2], F32, tag="mv")
                nc.vector.bn_aggr(out=mv, in_=st)
                # stats_all[:, i] = channel mean;  stats_all[:, 4+i] = E[x^2]
                nc.vector.tensor_copy(out=stats_all[:, i : i + 1], in_=mv[:, 0:1])
                nc.vector.tensor_mul(
                    out=stats_all[:, 4 + i : 5 + i], in0=mv[:, 0:1], in1=mv[:, 0:1]
                )
                nc.vector.tensor_add(
                    out=stats_all[:, 4 + i : 5 + i],
                    in0=stats_all[:, 4 + i : 5 + i],
                    in1=mv[:, 1:2],
                )
            else:
                sq_scr = qkvp.tile([128, N], BF16, tag="qkv0", name=f"sq_scr{i}")
                sx2 = spool.tile([128, 1], F32, tag="sx2", name=f"sx2_{i}")
                nc.scalar.activation(
                    out=sq_scr,
                    in_=xti,
                    func=mybir.ActivationFunctionType.Square,
                    accum_out=sx2,
                )
                sx1 = spool.tile([128, 1], F32, tag="sx1", name=f"sx1_{i}")
                nc.scalar.activation(
                    out=xti,
                    in_=xti,
                    func=mybir.ActivationFunctionType.Identity,
                    accum_out=sx1,
                )
                nc.vector.tensor_scalar_mul(
                    out=stats_all[:, i : i + 1], in0=sx1, scalar1=1.0 / N
                )
                nc.vector.tensor_scalar_mul(
                    out=stats_all[:, 4 + i : 5 + i], in0=sx2, scalar1=1.0 / N
                )

        # ---- cross-partition group aggregation via PE ----
        stats16 = gn.tile([128, 8], BF16, tag="stats16")
        nc.vector.tensor_copy(out=stats16, in_=stats_all)
        ps_t = ps_mm2.tile([8, 8], F32, tag="mm2")
        nc.tensor.matmul(ps_t, lhsT=mgrp_sb, rhs=stats16, start=True, stop=True)
        gs = gn.tile([8, 8], F32, tag="gs8")
        nc.scalar.mul(out=gs, in_=ps_t, mul=1.0 / 16.0)
        # gvals cols 0..3 = group mean per x-tile, cols 4..7 = group rstd
        gvals = gn.tile([8, 8], F32, tag="gvals")
        nc.vector.tensor_copy(out=gvals[:, 0:4], in_=gs[:, 0:4])
        varg = gn.tile([8, 4], F32, tag="varg")
        nc.vector.tensor_mul(out=varg, in0=gs[:, 0:4], in1=gs[:, 0:4])  # mu^2
        nc.vector.tensor_sub(out=varg, in0=gs[:, 4:8], in1=varg)  # var
        nc.scalar.activation(
            out=varg,
            in_=varg,
            func=mybir.ActivationFunctionType.Sqrt,
            bias=eps_sb[0:8, :],
        )
        nc.vector.reciprocal(out=gvals[:, 4:8], in_=varg)  # rstd
        gvals16 = gn.tile([8, 8], BF16, tag="gvals16")
        nc.vector.tensor_copy(out=gvals16, in_=gvals)
        ps_t2 = ps_mm2.tile([128, 8], F32, tag="mm2")
        nc.tensor.matmul(ps_t2, lhsT=mgrpT_sb, rhs=gvals16, start=True, stop=True)
        sc_all = gn.tile([128, 8], F32, tag="scall")
        nc.vector.tensor_copy(out=sc_all, in_=ps_t2)

        # ---- fold normalization into the weights instead of applying to x:
        # qkv = (W' * rstd[cin]) @ x_raw + (b' - (W'*rstd) @ mu) ----
        wts = []
        nmu = gn.tile([128, 4], BF16, tag="nmu")
        for kt in range(4):
            wt_s = qkvp.tile([128, 3 * CH], BF16, tag=f"wts{kt}", name=f"wts{kt}")
            nc.vector.tensor_scalar_mul(
                out=wt_s, in0=w_tiles[kt], scalar1=sc_all[:, 4 + kt : 5 + kt]
            )
            wts.append(wt_s)
            nc.vector.tensor_scalar_mul(
                out=nmu[:, kt : kt + 1],
                in0=sc_all[:, kt : kt + 1],
                scalar1=-1.0,
            )
        beff = gn.tile([128, 3], F32, tag="beff")
        for j in range(3):
            ps_b = ps_mm2.tile([128, 1], F32, tag="mm2", name=f"ps_b{j}")
            for kt in range(4):
                nc.tensor.matmul(
                    ps_b,
                    lhsT=wts[kt][:, j * 128 : (j + 1) * 128],
                    rhs=nmu[:, kt : kt + 1],
                    start=(kt == 0),
                    stop=(kt == 3),
                )
            nc.vector.tensor_add(
                out=beff[:, j : j + 1], in0=bq_sb[:, j : j + 1], in1=ps_b
            )

        # ---- qkv = W' @ xn + b', chunk-major, v first so vT transposes
        # (serial 1.3us DMA-xbar ops) start as early as possible ----
        qkv_sb = [None, None, None]
        for j in range(3):
            qkv_sb[j] = qkvp.tile([128, N], BF16, tag=f"qkv{j}", name=f"qkv{j}")
        q_sb, k_sb, v_sb = qkv_sb
        vT = qkvp.tile([128, NST, 128], BF16, tag="vT")

        def qkv_pair(j, c2):
            # 1024 output cols (two 512 psum banks), stationary shared
            # between the two halves per kt to amortize LDWEIGHTS
            ps = ps_sc.tile(
                [128, 1024], F32, tag="sc", name=f"qps{j}_{c2}"
            )
            for kt in range(4):
                for half in range(2):
                    nc.tensor.matmul(
                        ps[:, 512 * half : 512 * (half + 1)],
                        lhsT=wts[kt][:, j * 128 : (j + 1) * 128],
                        rhs=xt[kt][
                            :, 1024 * c2 + 512 * half : 1024 * c2 + 512 * (half + 1)
                        ],
                        start=(kt == 0),
                        stop=(kt == 3),
                    )
            nc.scalar.activation(
                out=qkv_sb[j][:, 1024 * c2 : 1024 * (c2 + 1)],
                in_=ps,
                func=mybir.ActivationFunctionType.Identity,
                bias=beff[:, j : j + 1],
            )

        # ---- pipelined rounds: scores+exp(r) interleaved with attn@v(r-1).
        # Round 0: k fully + first q pair upfront, then v/q pairs and vT
        # transposes spread between the score blocks. ----
        for c2 in range(4):
            qkv_pair(1, c2)  # k
        qkv_pair(0, 0)  # q cols 0..1023 (all that chunk-0 scores need)
        ets_prev = None
        for r in range(NCHUNK + 1):
            t0 = r * TCHUNK
            tp = (r - 1) * TCHUNK

            if r >= 1:
                # Z add-tree for chunk r-1 over the 16 pair tiles, emitted up
                # front (vector runs it while PE+ACT stream the st loop);
                # FD=2048 ops, in-place reduction on 8 temps
                tt = []
                for j in range(8):
                    t_ = trpool.tile([128, 2, TCHUNK], F16, tag="trv", name=f"t{j}")
                    nc.vector.tensor_add(
                        out=t_, in0=ets_prev[2 * j], in1=ets_prev[2 * j + 1]
                    )
                    tt.append(t_)
                for span in (2, 4, 8):
                    for j in range(0, 8, span):
                        nc.vector.tensor_add(
                            out=tt[j], in0=tt[j], in1=tt[j + span // 2]
                        )
                ps_h = [
                    ps_acc.tile([128, 512], F32, tag="acc", name=f"ps_h{i}")
                    for i in range(2)
                ]

            ets = []
            for stt in range(NST):
                if r == 0 and stt % 8 == 0:
                    c2 = stt // 8
                    if c2 >= 1:
                        qkv_pair(0, c2)  # next q pair
                    qkv_pair(2, c2)  # v columns for these s tiles
                    for st2 in range(8 * c2, 8 * c2 + 8):
                        eng = nc.sync if st2 % 2 == 0 else nc.scalar
                        eng.dma_start_transpose(
                            vT[:, st2, :], v_sb[:, 128 * st2 : 128 * (st2 + 1)]
                        )
                if r < NCHUNK:
                    ps = ps_sc.tile([128, TCHUNK], F32, tag="sc")
                    kslice = k_sb[:, 128 * stt : 128 * (stt + 1)]
                    for hh in range(2):
                        nc.tensor.matmul(
                            ps[:, 512 * hh : 512 * (hh + 1)],
                            lhsT=kslice,
                            rhs=q_sb[:, t0 + 512 * hh : t0 + 512 * (hh + 1)],
                            start=True,
                            stop=True,
                        )
                    if stt % 2 == 0:
                        et = epool.tile([128, 2, TCHUNK], BF16, tag="et")
                        ets.append(et)
                    nc.scalar.activation(
                        out=ets[stt // 2][:, stt % 2, :],
                        in_=ps,
                        func=mybir.ActivationFunctionType.Exp,
                    )
                if r >= 1:
                    ep = ets_prev[stt // 2]
                    for hh in range(2):
                        nc.tensor.matmul(
                            ps_h[hh],
                            lhsT=vT[:, stt, :],
                            rhs=ep[:, stt % 2, 512 * hh : 512 * (hh + 1)],
                            start=(stt == 0),
                            stop=(stt == NST - 1),
                        )

            if r >= 1:
                # finish Z tree, Z matmul, ship Z
                esum = espool.tile([128, TCHUNK], F16, tag="esum")
                nc.vector.tensor_add(out=esum, in0=tt[0][:, 0, :], in1=tt[0][:, 1, :])
                zrow = zpool.tile([1, TCHUNK], F32, tag="zrow")
                for hh in range(2):
                    ps_z = ps_mm2.tile([1, 512], F32, tag="mm2")
                    nc.tensor.matmul(
                        ps_z,
                        lhsT=ones_col,
                        rhs=esum[:, 512 * hh : 512 * (hh + 1)],
                        start=True,
                        stop=True,
                    )
                    nc.vector.tensor_copy(
                        out=zrow[:, 512 * hh : 512 * (hh + 1)], in_=ps_z
                    )
                nc.sync.dma_start(out=zout[:, tp : tp + TCHUNK], in_=zrow)

                # h_unnorm, proj, store
                for hh in range(2):
                    h_sb = hpool.tile([128, 512], BF16, tag="h")
                    nc.vector.tensor_copy(out=h_sb, in_=ps_h[hh])
                    for ot in range(4):
                        ps_p = ps_mm2.tile([128, 512], F32, tag="mm2")
                        nc.tensor.matmul(
                            ps_p,
                            lhsT=wp_sb[:, 128 * ot : 128 * (ot + 1)],
                            rhs=h_sb,
                            start=True,
                            stop=True,
                        )
                        ob = opool.tile([128, 512], F32, tag="osb")
                        nc.vector.tensor_copy(out=ob, in_=ps_p)
                        nc.sync.dma_start(
                            out=partial[
                                128 * ot : 128 * (ot + 1),
                                tp + 512 * hh : tp + 512 * (hh + 1),
                            ],
                            in_=ob,
                        )
            ets_prev = ets if r < NCHUNK else None

    if not nc.is_finalized():
        nc.finalize()
    return nc


_NC_CACHE = None


def _get_nc():
    global _NC_CACHE
    if _NC_CACHE is None:
        _NC_CACHE = build_program()
    return _NC_CACHE


def kernel(x, norm_w, norm_b, w_qkv, w_proj, b_proj):
    global LAST_RESULT
    x = np.asarray(x, dtype=np.float32)
    norm_w = np.asarray(norm_w, dtype=np.float32)
    norm_b = np.asarray(norm_b, dtype=np.float32)
    w_qkv = np.asarray(w_qkv, dtype=np.float32)
    w_proj = np.asarray(w_proj, dtype=np.float32)
    b_proj = np.asarray(b_proj, dtype=np.float32)

    s1 = 1.0 / math.sqrt(math.sqrt(CH))
    bf16 = ml_dtypes.bfloat16
    mgrp = (np.arange(128)[:, None] // 16 == np.arange(8)[None, :]).astype(bf16)
    in_maps = []
    for core in range(NCORES):
        b, h = divmod(core, NH)
        # reference layout: head h of batch b uses w_qkv rows
        # [384h:384h+128] (q), [384h+128:384h+256] (k), [384h+256:384h+384] (v)
        rows = w_qkv[384 * h : 384 * (h + 1)]  # (384, 512)
        wfold = rows * norm_w[None, :]  # fold GroupNorm gamma
        bias = rows @ norm_b  # fold GroupNorm beta
        scale_vec = np.concatenate(
            [np.full(128, s1), np.full(128, s1), np.ones(128)]
        ).astype(np.float32)
        wfold = wfold * scale_vec[:, None]
        bias = bias * scale_vec
        wqkvT = np.ascontiguousarray(wfold.T.reshape(4, 128, 384).astype(bf16))
        bqkv = np.ascontiguousarray(bias.reshape(3, 128).T.astype(np.float32))
        wprojT = np.ascontiguousarray(
            w_proj[:, 128 * h : 128 * (h + 1)].T.astype(bf16)
        )
        x16 = np.ascontiguousarray(x[b].reshape(C, N).astype(bf16))
        in_maps.append(
            {
                "x16": x16,
                "wqkvT": wqkvT,
                "bqkv": bqkv,
                "wprojT": wprojT,
                "mgrp": mgrp,
                "mgrpT": np.ascontiguousarray(mgrp.T),
            }
        )

    nc = _get_nc()
    res = run_bass_kernel_spmd(
        nc,
        in_maps,
        list(range(NCORES)),
        trace=TRACE,
        trace_cores=TRACE_CORES if TRACE else None,
    )
    LAST_RESULT = res

    out = np.empty((B, C, N), dtype=np.float32)
    for b in range(B):
        acc = x[b].reshape(C, N) + b_proj[:, None]
        for h in range(NH):
            r = res.results[4 * b + h]
            acc = acc + r["partial"] / r["zout"]
        out[b] = acc
    return out.reshape(B, C, 64, 64)
